# revision 11
# baseline (speedup 1.0000x reference)
"""Trainium2 Bass kernel for a full MHA block (QKV proj -> masked softmax
attention -> output proj -> residual + LayerNorm), returning (y, atten).

Sharding: 8 cores, core c owns batch b=c//4 and query rows [512*(c%4), 512*(c%4+1)).
Each core computes all 16 heads for its query rows; K/V projections for its batch
are computed on-core (replicated across the 4 cores sharing a batch), so no
cross-core collectives are needed.
"""

import sys

for p in ("/opt/trn_rl_repo",):
    if p not in sys.path:
        sys.path.insert(0, p)

import numpy as np

import concourse.bass as bass
import concourse.tile as tile
from concourse import bacc, mybir
from concourse.masks import make_identity
from concourse.bass_utils import run_bass_kernel_spmd

F32 = mybir.dt.float32
BF16 = mybir.dt.bfloat16
U8 = mybir.dt.uint8
AF = mybir.ActivationFunctionType
OP = mybir.AluOpType

D = 1024        # d_model
H = 16          # heads
DH = 64         # head dim
L = 2048        # seq len
LQ = 512        # query rows per core
B = 2
NCORES = 8
NEG = -1e9
EPS = 1e-5
SCALE = 0.125   # 1/sqrt(64)

QT = LQ // 128          # 4 query tiles per core
KO = D // 128           # 8 contraction tiles over d_model
NPAIR = H // 2          # 8 head pairs
KT = L // 128           # 16 key tiles


def build_kernel():
    nc = bacc.Bacc("TRN2", target_bir_lowering=False, debug=False, num_devices=NCORES)

    # ---- DRAM I/O (per-core shapes) ----
    xq = nc.dram_tensor("xq", [LQ, D], F32, kind="ExternalInput").ap()
    xk = nc.dram_tensor("xk", [L, D], F32, kind="ExternalInput").ap()
    xv = nc.dram_tensor("xv", [L, D], F32, kind="ExternalInput").ap()
    msk = nc.dram_tensor("msk", [LQ, L], U8, kind="ExternalInput").ap()
    wq = nc.dram_tensor("wq", [D, D], F32, kind="ExternalInput").ap()
    wk = nc.dram_tensor("wk", [D, D], F32, kind="ExternalInput").ap()
    wv = nc.dram_tensor("wv", [D, D], F32, kind="ExternalInput").ap()
    wo = nc.dram_tensor("wo", [D, D], F32, kind="ExternalInput").ap()
    bq = nc.dram_tensor("bq", [1, D], F32, kind="ExternalInput").ap()
    bk = nc.dram_tensor("bk", [1, D], F32, kind="ExternalInput").ap()
    bv = nc.dram_tensor("bv", [1, D], F32, kind="ExternalInput").ap()
    bo = nc.dram_tensor("bo", [1, D], F32, kind="ExternalInput").ap()
    gamma = nc.dram_tensor("gamma", [1, D], F32, kind="ExternalInput").ap()
    beta = nc.dram_tensor("beta", [1, D], F32, kind="ExternalInput").ap()

    attn_out = nc.dram_tensor("attn_out", [H, LQ, L], F32, kind="ExternalOutput").ap()
    y_out = nc.dram_tensor("y_out", [LQ, D], F32, kind="ExternalOutput").ap()

    # DRAM scratch for spilled kT / v (per-pair layout)
    kTd = nc.dram_tensor("kTd", [NPAIR, 128, L], F32, kind="Internal").ap()
    vd = nc.dram_tensor("vd", [NPAIR, KT, 128, 128], F32, kind="Internal").ap()

    with tile.TileContext(nc) as tc:
      with tc.tile_pool(name="const", bufs=1) as const, \
           tc.tile_pool(name="per", bufs=1) as per, \
           tc.tile_pool(name="ps_big", bufs=2, space="PSUM") as ps_big, \
           tc.tile_pool(name="ps_tp", bufs=2, space="PSUM") as ps_tp:

        # ---- constants ----
        ident = const.tile([128, 128], F32, tag="ident")
        make_identity(nc, ident[:])
        idneg = const.tile([128, 128], BF16, tag="idneg")
        nc.gpsimd.memset(idneg[:], 0.0)
        nc.gpsimd.affine_select(
            out=idneg[:], in_=idneg[:], compare_op=OP.not_equal, fill=NEG,
            base=0, pattern=[[-1, 128]], channel_multiplier=1,
        )
        ones_col = const.tile([1, 128], F32, tag="ones_col")
        nc.vector.memset(ones_col[:], 1.0)
        ones_row = const.tile([1, 512], F32, tag="ones_row")
        nc.vector.memset(ones_row[:], 1.0)

        # persistent across phases
        qT = per.tile([128, NPAIR, LQ], F32, tag="qT")
        maskf = per.tile([128, QT, L], BF16, tag="maskf")
        ctx_sb = per.tile([128, QT, D], F32, tag="ctx")

        # ---- helpers ----
        def transpose_rows(src_rows, x_t, n_ko=KO):
            """src_rows: sbuf [128, n_ko*128]; x_t: sbuf [128, n_ko, 128]."""
            for g in range(0, n_ko, 4):
                pt = ps_tp.tile([128, 512], F32, tag="ps_tp")
                n4 = min(4, n_ko - g)
                for i in range(n4):
                    nc.tensor.transpose(
                        pt[:, i * 128:(i + 1) * 128],
                        src_rows[:, (g + i) * 128:(g + i + 1) * 128],
                        ident[:],
                    )
                nc.any.tensor_copy(
                    x_t[:, g:g + n4, :],
                    pt[:, :n4 * 128].rearrange("p (f m) -> p f m", f=n4),
                )

        # =============== phase 1: projections ===============
        with tc.tile_pool(name="p1", bufs=1) as p1:

            def load_w(ap_, name):
                t = p1.tile([128, KO, D], F32, tag="wmat", name=name)
                nc.sync.dma_start(t[:], ap_.rearrange("(ko ki) m -> ki ko m", ki=128))
                return t

            def load_brow(ap_, name):
                t = p1.tile([1, D], F32, tag="brow", name=name)
                nc.sync.dma_start(t[:], ap_)
                return t

            # -- 1a: qT (all pairs) --
            wq_sb = load_w(wq, "wq")
            bq_sb = load_brow(bq, "bq")
            for qt in range(QT):
                xr = p1.tile([128, D], F32, tag="xrow")
                nc.sync.dma_start(xr[:], xq[qt * 128:(qt + 1) * 128, :])
                xt_q = p1.tile([128, KO, 128], F32, tag="xt128")
                transpose_rows(xr, xt_q)
                for p in range(NPAIR):
                    pb = ps_big.tile([128, 1024], F32, tag="ps_big")
                    for ko in range(KO):
                        nc.tensor.matmul(pb[:, :128], wq_sb[:, ko, p * 128:(p + 1) * 128],
                                         xt_q[:, ko, :], start=(ko == 0), stop=False)
                    nc.tensor.matmul(pb[:, :128], bq_sb[:, p * 128:(p + 1) * 128],
                                     ones_row[:, :128], start=False, stop=True)
                    nc.any.tensor_copy(qT[:, p, qt * 128:(qt + 1) * 128], pb[:, :128])

            # -- 1b: kT -> DRAM spill --
            wk_sb = load_w(wk, "wk")
            bk_sb = load_brow(bk, "bk")
            for c in range(4):  # 512-wide chunks of L
                xt_k = p1.tile([128, KO, 512], F32, tag="xt512")
                for rt in range(4):
                    xr = p1.tile([128, D], F32, tag="xrow")
                    nc.sync.dma_start(
                        xr[:], xk[(c * 4 + rt) * 128:(c * 4 + rt + 1) * 128, :])
                    transpose_rows(xr, xt_k[:, :, rt * 128:(rt + 1) * 128])
                for p in range(NPAIR):
                    pb = ps_big.tile([128, 1024], F32, tag="ps_big")
                    for ko in range(KO):
                        nc.tensor.matmul(pb[:, :512], wk_sb[:, ko, p * 128:(p + 1) * 128],
                                         xt_k[:, ko, :], start=(ko == 0), stop=False)
                    nc.tensor.matmul(pb[:, :512], bk_sb[:, p * 128:(p + 1) * 128],
                                     ones_row[:], start=False, stop=True)
                    kt_tmp = p1.tile([128, 512], F32, tag="kt_tmp")
                    nc.any.tensor_copy(kt_tmp[:], pb[:, :512])
                    nc.sync.dma_start(kTd[p, :, c * 512:(c + 1) * 512], kt_tmp[:])

            # -- 1c: v -> DRAM spill (per-pair layout) --
            wv_sb = load_w(wv, "wv")
            bv_sb = load_brow(bv, "bv")
            for rt in range(KT):
                xr = p1.tile([128, D], F32, tag="xrow")
                nc.sync.dma_start(xr[:], xv[rt * 128:(rt + 1) * 128, :])
                xt_v = p1.tile([128, KO, 128], F32, tag="xt128")
                transpose_rows(xr, xt_v)
                for c2 in range(2):
                    pb = ps_big.tile([128, 1024], F32, tag="ps_big")
                    for ko in range(KO):
                        nc.tensor.matmul(pb[:, :512], xt_v[:, ko, :],
                                         wv_sb[:, ko, c2 * 512:(c2 + 1) * 512],
                                         start=(ko == 0), stop=False)
                    nc.tensor.matmul(pb[:, :512], ones_col[:],
                                     bv_sb[:, c2 * 512:(c2 + 1) * 512],
                                     start=False, stop=True)
                    v_tmp = p1.tile([128, 512], F32, tag="v_tmp")
                    nc.any.tensor_copy(v_tmp[:], pb[:, :512])
                    nc.sync.dma_start(
                        vd[c2 * 4:(c2 + 1) * 4, rt].rearrange("f p m -> p f m"),
                        v_tmp[:].rearrange("p (f m) -> p f m", f=4),
                    )

        # =============== phase 2: attention ===============
        with tc.tile_pool(name="p2", bufs=2) as p2, \
             tc.tile_pool(name="p2s", bufs=4) as p2s, \
             tc.tile_pool(name="ps_ctx", bufs=2, space="PSUM") as ps_ctx:

            # mask: load u8 per q-tile, convert to bf16 (values 0/1)
            for qt in range(QT):
                mu8 = p2.tile([128, L], U8, tag="mu8")
                nc.sync.dma_start(
                    mu8[:], msk[qt * 128:(qt + 1) * 128, :])
                nc.vector.tensor_copy(maskf[:, qt], mu8[:])

            for p in range(NPAIR):
                kt_p = p2.tile([128, L], F32, tag="kt_pair")
                nc.sync.dma_start(kt_p[:], kTd[p])
                v_p = p2.tile([128, KT, 128], F32, tag="v_pair")
                nc.sync.dma_start(v_p[:], vd[p].rearrange("lt p m -> p lt m"))
                for qt in range(QT):
                    for hh in range(2):
                        h = 2 * p + hh
                        m_sb = p2.tile([128, L], F32, tag="m_sb")
                        rs = p2s.tile([128, 1], F32, tag="rs")
                        rs2 = p2s.tile([128, 1], F32, tag="rs2")
                        for half in range(2):
                            pb = ps_big.tile([128, 1024], F32, tag="ps_big")
                            for j in range(2):
                                sl = slice(j * 512, (j + 1) * 512)
                                ksl = slice(half * 1024 + j * 512,
                                            half * 1024 + (j + 1) * 512)
                                nc.tensor.matmul(
                                    pb[:, sl],
                                    qT[hh * 64:(hh + 1) * 64, p, qt * 128:(qt + 1) * 128],
                                    kt_p[hh * 64:(hh + 1) * 64, ksl],
                                    start=True, stop=False)
                                nc.tensor.matmul(
                                    pb[:, sl], idneg[:], maskf[:, qt, ksl],
                                    start=False, stop=True)
                            nc.scalar.activation(
                                m_sb[:, half * 1024:(half + 1) * 1024], pb[:],
                                AF.Exp, scale=SCALE,
                                accum_out=(rs if half == 0 else rs2)[:],
                            )
                        rsum = p2s.tile([128, 1], F32, tag="rsum")
                        nc.vector.tensor_tensor(rsum[:], rs[:], rs2[:], OP.add)
                        rinv = p2s.tile([128, 1], F32, tag="rinv")
                        nc.vector.reciprocal(rinv[:], rsum[:])
                        # normalize in place -> m_sb becomes atten
                        nc.vector.tensor_scalar_mul(m_sb[:], m_sb[:], rinv[:])
                        nc.sync.dma_start(
                            attn_out[h, qt * 128:(qt + 1) * 128, :], m_sb[:])
                        # transpose atten (PE + copies), then ctx matmuls
                        mt = p2.tile([128, KT, 128], F32, tag="mt_full")
                        for kc in range(4):
                            pt = ps_tp.tile([128, 512], F32, tag="ps_tp")
                            for i in range(4):
                                nc.tensor.transpose(
                                    pt[:, i * 128:(i + 1) * 128],
                                    m_sb[:, (kc * 4 + i) * 128:(kc * 4 + i + 1) * 128],
                                    ident[:])
                            dst = mt[:, kc * 4:(kc + 1) * 4, :]
                            src = pt[:].rearrange("p (f m) -> p f m", f=4)
                            if kc % 2 == 0:
                                nc.vector.tensor_copy(dst, src)
                            else:
                                nc.scalar.copy(dst, src)
                        pc = ps_ctx.tile([128, 64], F32, tag="ps_ctx")
                        for kt_i in range(KT):
                            nc.tensor.matmul(
                                pc[:], mt[:, kt_i, :],
                                v_p[:, kt_i, hh * 64:(hh + 1) * 64],
                                start=(kt_i == 0), stop=(kt_i == KT - 1))
                        nc.any.tensor_copy(
                            ctx_sb[:, qt, h * 64:(h + 1) * 64], pc[:])

        # =============== phase 3: out-proj + residual + LayerNorm ===============
        with tc.tile_pool(name="p3", bufs=1) as p3, \
             tc.tile_pool(name="p3t", bufs=4) as p3t, \
             tc.tile_pool(name="p3s", bufs=4) as p3s:

            wo_sb = p3.tile([128, KO, D], F32, tag="wmat3")
            nc.sync.dma_start(wo_sb[:], wo.rearrange("(ko ki) m -> ki ko m", ki=128))
            bo_sb = p3.tile([1, D], F32, tag="bo3")
            nc.sync.dma_start(bo_sb[:], bo)

            # broadcast gamma/beta to [128, D] via PE ones-trick
            bcast = {}
            for name, ap_ in (("gamma", gamma), ("beta", beta)):
                row = p3t.tile([1, D], F32, tag="vrow")
                nc.sync.dma_start(row[:], ap_)
                dst = p3.tile([128, D], F32, tag=f"bc_{name}")
                for c2 in range(2):
                    pb = ps_big.tile([128, 1024], F32, tag="ps_big")
                    nc.tensor.matmul(pb[:, :512], ones_col[:],
                                     row[:, c2 * 512:(c2 + 1) * 512],
                                     start=True, stop=True)
                    nc.any.tensor_copy(dst[:, c2 * 512:(c2 + 1) * 512], pb[:, :512])
                bcast[name] = dst

            for qt in range(QT):
                ctxT = p3t.tile([128, KO, 128], F32, tag="ctxT")
                transpose_rows(ctx_sb[:, qt], ctxT)
                po = ps_big.tile([128, 1024], F32, tag="ps_big")
                for c2 in range(2):
                    sl = slice(c2 * 512, (c2 + 1) * 512)
                    for ko in range(KO):
                        nc.tensor.matmul(po[:, sl], ctxT[:, ko, :], wo_sb[:, ko, sl],
                                         start=(ko == 0), stop=False)
                    nc.tensor.matmul(po[:, sl], ones_col[:], bo_sb[:, sl],
                                     start=False, stop=True)
                xr = p3t.tile([128, D], F32, tag="xrow3")
                nc.sync.dma_start(xr[:], xq[qt * 128:(qt + 1) * 128, :])
                x_sb = p3t.tile([128, D], F32, tag="lnt", name="x_sb")
                xsum = p3s.tile([128, 1], F32, tag="xsum")
                nc.vector.scalar_tensor_tensor(
                    x_sb[:], po[:], 0.0, xr[:], OP.bypass, OP.add, accum_out=xsum[:])
                negmu = p3s.tile([128, 1], F32, tag="negmu")
                nc.vector.tensor_scalar_mul(negmu[:], xsum[:], -1.0 / D)
                t1 = p3t.tile([128, D], F32, tag="lnt", name="t1")
                nc.vector.tensor_scalar_add(t1[:], x_sb[:], negmu[:])
                sq = p3t.tile([128, D], F32, tag="lnt", name="sq")
                sqs = p3s.tile([128, 1], F32, tag="sqs")
                nc.scalar.activation(sq[:], t1[:], AF.Square, accum_out=sqs[:])
                vv = p3s.tile([128, 1], F32, tag="vv")
                nc.vector.tensor_scalar(vv[:], sqs[:], 1.0 / D, EPS, OP.mult, OP.add)
                vr = p3s.tile([128, 1], F32, tag="vr")
                nc.vector.reciprocal(vr[:], vv[:])
                s_t = p3s.tile([128, 1], F32, tag="s_t")
                nc.scalar.activation(s_t[:], vr[:], AF.Sqrt)
                y1 = p3t.tile([128, D], F32, tag="lnt", name="y1")
                nc.vector.scalar_tensor_tensor(
                    y1[:], t1[:], s_t[:], bcast["gamma"][:], OP.mult, OP.mult)
                y2 = p3t.tile([128, D], F32, tag="lnt", name="y2")
                nc.vector.tensor_tensor(y2[:], y1[:], bcast["beta"][:], OP.add)
                nc.sync.dma_start(y_out[qt * 128:(qt + 1) * 128, :], y2[:])

    nc.compile()
    return nc


_NC = None


def _get_nc():
    global _NC
    if _NC is None:
        _NC = build_kernel()
    return _NC


def make_in_maps(Q, K, V, atten_mask, Wq, bq, Wk, bk, Wv, bv, Wo, bo, gamma, beta):
    Q = np.asarray(Q, np.float32)
    K = np.asarray(K, np.float32)
    V = np.asarray(V, np.float32)
    mask_u8 = np.ascontiguousarray(np.asarray(atten_mask)).view(np.uint8)
    shared = {
        "wq": np.ascontiguousarray(np.asarray(Wq, np.float32)),
        "wk": np.ascontiguousarray(np.asarray(Wk, np.float32)),
        "wv": np.ascontiguousarray(np.asarray(Wv, np.float32)),
        "wo": np.ascontiguousarray(np.asarray(Wo, np.float32)),
        "bq": np.asarray(bq, np.float32).reshape(1, D),
        "bk": np.asarray(bk, np.float32).reshape(1, D),
        "bv": np.asarray(bv, np.float32).reshape(1, D),
        "bo": np.asarray(bo, np.float32).reshape(1, D),
        "gamma": np.asarray(gamma, np.float32).reshape(1, D),
        "beta": np.asarray(beta, np.float32).reshape(1, D),
    }
    in_maps = []
    for c in range(NCORES):
        b, qb = divmod(c, 4)
        sl = slice(qb * LQ, (qb + 1) * LQ)
        m = dict(shared)
        m["xq"] = np.ascontiguousarray(Q[b, sl])
        m["xk"] = np.ascontiguousarray(K[b])
        m["xv"] = np.ascontiguousarray(V[b])
        m["msk"] = np.ascontiguousarray(mask_u8[b, sl])
        in_maps.append(m)
    return in_maps


def kernel(Q, K, V, atten_mask, Wq, bq, Wk, bk, Wv, bv, Wo, bo, gamma, beta,
           _return_results=False, **_run_kwargs):
    nc = _get_nc()
    in_maps = make_in_maps(Q, K, V, atten_mask, Wq, bq, Wk, bk, Wv, bv, Wo, bo,
                           gamma, beta)
    res = run_bass_kernel_spmd(nc, in_maps, core_ids=list(range(NCORES)),
                               **_run_kwargs)
    y = np.empty((B, L, D), np.float32)
    atten = np.empty((B, H, L, L), np.float32)
    for c in range(NCORES):
        b, qb = divmod(c, 4)
        sl = slice(qb * LQ, (qb + 1) * LQ)
        out = res.results[c]
        y[b, sl] = out["y_out"]
        atten[b, :, sl, :] = out["attn_out"]
    if _return_results:
        return (y, atten), res
    return (y, atten)
